# revision 39
# baseline (speedup 1.0000x reference)
"""Disparity estimation loss kernel for Trainium2 (Bass/Tile), 8-core SPMD.

Reference computation (per pixel over the D=192 disparity axis):
    prob    = softmax(cost_volume, axis=D)
    mean    = sum(prob * d)
    var     = sum(prob * (d - mean)^2) = E[d^2] - mean^2
    logvar  = log(var + 1e-6)
Outputs: (mean [B,H,W], logvar [B,H,W]) both f32.

Strategy: shard H across 8 cores (H=256 -> 32 rows/core). All reductions are
along D which stays local. Per core, 16-h-row supergroups (2 per b):
  - Three DMA queues stream inputs concurrently (a single queue tops out
    well below the per-core HBM rate): SP HWDGE ring carries cv0 (d 0..127,
    [128, 16*512] f32, 4 MiB) of even supergroups, ACT HWDGE ring cv0 of
    odd supergroups, SWDGE the chunk1 slabs (d 128..191, two 64-partition
    slabs per supergroup, slab p = h rows h0+8p..h0+8p+8) casting f32->f16
    inline (SWDGE-only feature). All APs are simple single-level patterns
    (complex APs cost ~8us of sequencer time per HWDGE trigger). All
    triggers are hoisted to the top of each b so prefetch never queues
    behind compute.
  - exp on ScalarE -> fp16 (no max subtraction: inputs are N(0,1)), split
    per supergroup into a cv0 part and a chunk1 part so the serial tail
    after the last DMA is one small exp, not the whole supergroup.
  - TensorE matmuls contract over D: exp tile [D, 128 w-cols] stationary
    (fp16), weight columns [1, d, d^2_hi, d^2_lo] moving -> PSUM groups
    [128 w, 4]. d^2 split into exact-fp16 hi/lo bytes.
  - VectorE batched finalize (mean/var) per supergroup; Ln + PE transposes
    + output stores deferred to end-of-b (2 ACT table switches per b, and
    store triggers on the ACT HWDGE ring where their compute dependencies
    cannot block input prefetch).
"""

import os
import sys

for _p in ("/opt/trn_rl_repo", "/root/.axon_site/_ro/trn_rl_repo"):
    if os.path.isdir(_p) and _p not in sys.path:
        sys.path.insert(0, _p)

import ml_dtypes
import numpy as np

import concourse.bacc as bacc
import concourse.bass as bass
import concourse.tile as tile
from concourse import mybir
from concourse.bass_utils import run_bass_kernel_spmd
from concourse.masks import make_identity

B, D, H, W = 4, 192, 256, 512
N_CORES = 8
HL = H // N_CORES  # 32 h-rows per core
F32 = mybir.dt.float32
F16 = mybir.dt.float16

# knobs (test.py may flip these before calling kernel())
TRACE = False
LAST_RESULT = None


def _make_weights() -> np.ndarray:
    """[128, 12] fp16 weight matrix; every entry is exactly representable.

    cols 0:4  -> d-chunk0 (d = row p):        [1, d, hi(d^2), lo(d^2)]  (fp16)
    cols 4:12 -> d-chunk1 (two slabs stacked on partitions):
       rows 0:64   (slab lo, d = 128+p):      [1, d, hi, lo, 0, 0, 0, 0]
       rows 64:128 (slab hi, d = 64+p):       [0, 0, 0, 0, 1, d, hi, lo]
    where hi = d^2 >> 8 (<=142), lo = d^2 & 255 — both exact in fp16.
    """
    wk = np.zeros((128, 12), dtype=np.float64)

    def cols(d):
        dsq = (d.astype(np.int64)) ** 2
        return 1.0, d, (dsq >> 8).astype(np.float64), (dsq & 255).astype(np.float64)

    p = np.arange(128, dtype=np.int64)
    wk[:, 0], wk[:, 1], wk[:, 2], wk[:, 3] = cols(p)
    c = cols(128 + p[:64])
    for k in range(4):
        wk[:64, 4 + k] = c[k]
    c = cols(64 + p[64:])
    for k in range(4):
        wk[64:, 8 + k] = c[k]
    return wk.astype(np.float16)


def build_core_kernel():
    """Build the per-core Bass module (identical program on all 8 cores)."""
    nc = bacc.Bacc("TRN2", target_bir_lowering=False, debug=False)
    x = nc.dram_tensor("x", [B, D, HL, W], F32, kind="ExternalInput")
    wk = nc.dram_tensor("wk", [128, 12], F16, kind="ExternalInput")
    mean_o = nc.dram_tensor("mean", [B, HL, W], F32, kind="ExternalOutput")
    logv_o = nc.dram_tensor("logvar", [B, HL, W], F32, kind="ExternalOutput")

    NG = 2  # supergroups per b, 16 h rows each
    GH = HL // NG  # 16
    CW = GH * W  # 8192 f32 cols of chunk0 per supergroup
    C1 = CW // 2  # 4096 cols of packed chunk1

    with tile.TileContext(nc) as tc:
        with (
            tc.tile_pool(name="cv", bufs=4) as cvp,
            tc.tile_pool(name="ex", bufs=1) as exp_p,
            tc.tile_pool(name="ex1", bufs=2) as exp1_p,
            tc.tile_pool(name="consts", bufs=1) as consts,
            tc.tile_pool(name="fin", bufs=3) as finp,
            tc.tile_pool(name="tmps", bufs=2) as tmpp,
            # outp depth 4: with 2, the DVE copy into a recycled output tile
            # waits for the PREVIOUS b's store DMA, which sits on SP behind
            # input triggers that wait on exp-fed buffer sems — a slack
            # spiral that cost ~25us per two b's.
            tc.tile_pool(name="outp", bufs=4) as outp,
            tc.tile_pool(name="psum", bufs=3, space="PSUM") as psp,
            tc.tile_pool(name="pst", bufs=2, space="PSUM") as pstp,
        ):
            wkt = consts.tile([128, 12], F16, tag="wk")
            nc.sync.dma_start(out=wkt, in_=wk[:, :])
            ident = consts.tile([128, 128], F32, tag="ident")
            make_identity(nc, ident)
            eps_t = consts.tile([128, 1], F32, tag="eps")
            nc.vector.memset(eps_t, 1e-6)

            # ---- input loading. Three queues with EQUAL byte shares stream
            # concurrently (the aggregate DMA rate rises with the number of
            # simultaneously-backlogged queues — SDMA engines are per-packet
            # latency-bound): ACT HWDGE ring takes cv0 of g0, triggered ONE
            # b AHEAD (ACT's triggers sit behind its exps, and at ~1/3 of
            # the aggregate rate a 4 MiB delivery needs most of a b of
            # lead), SP HWDGE ring cv0 of g1 (SP free-runs and carries the
            # last-consumed group), SWDGE the chunk1 slabs, casting
            # f32->f16 inline (SWDGE-only feature; halves SBUF footprint so
            # the cv pool can quad-buffer). cv0 is loaded as two 2 MiB
            # halves with matching split exps, so compute starts on the
            # first half while the second streams.
            def load_group(b, g):
                h0 = GH * g
                cv0 = cvp.tile([128, CW], F32, tag="cv0")
                c1f = cvp.tile([128, C1], F16, tag="c1f")
                # d 0..127 for h rows h0..h0+15 (2 x 2 MiB, simple APs),
                # one half per HWDGE ring: each group's cv0 arrives 2-way
                # parallel (the lookahead gives the ACT ring enough lead).
                # The very first group loads in 1 MiB quarters so the first
                # exp starts ~8us earlier (pipeline-fill is a pure tail
                # cost — nothing overlaps it).
                if b == 0 and g == 0:
                    for q in range(4):
                        eng = nc.sync if q < 2 else nc.scalar
                        eng.dma_start(
                            out=cv0[:, q * (CW // 4) : (q + 1) * (CW // 4)],
                            in_=x[b, 0:128, h0 + 4 * q : h0 + 4 * q + 4, :],
                        )
                else:
                    nc.sync.dma_start(
                        out=cv0[:, 0 : CW // 2], in_=x[b, 0:128, h0 : h0 + 8, :]
                    )
                    nc.scalar.dma_start(
                        out=cv0[:, CW // 2 : CW],
                        in_=x[b, 0:128, h0 + 8 : h0 + GH, :],
                    )
                # d 128..191, slab p: partitions 64p..64p+64 hold h rows
                # h0+8p..h0+8p+8 (1 MiB read each, single-level AP)
                for p in range(2):
                    nc.gpsimd.dma_start(
                        out=c1f[64 * p : 64 * p + 64, :],
                        in_=x[b, 128:192, h0 + 8 * p : h0 + 8 * p + 8, :],
                    )
                return cv0, c1f

            pending_stores = []
            loaded = {}
            loaded[(0, 0)] = load_group(0, 0)
            loaded[(0, 1)] = load_group(0, 1)
            for b in range(B):
                # one PSUM bank per b for chunk0 sums, one for chunk1 sums
                bankA = psp.tile([128, 512], F32, tag="bankA")
                bankB = psp.tile([128, 512], F32, tag="bankB")

                # trigger the NEXT b's loads here — a full b of lead for
                # every queue (the cv pool's bufs=4 rotation keeps the
                # buffer-free waits pointing at already-executed exps)
                if b + 1 < B:
                    loaded[(b + 1, 0)] = load_group(b + 1, 0)
                    loaded[(b + 1, 1)] = load_group(b + 1, 1)
                # previous b's output stores ride the SP ring BEHIND this
                # b's input triggers: their deps are long-ready so they
                # cannot stall input prefetch, and they cost the ACT engine
                # (the co-critical one) nothing.
                for dst, t, sb, sg in pending_stores:
                    nc.sync.dma_start(
                        out=dst[sb, GH * sg : GH * sg + GH].rearrange(
                            "h (c w) -> h c w", c=4
                        ),
                        in_=t,
                    )
                pending_stores = []

                fins = []
                for g in range(NG):
                    cv0, c1f = loaded.pop((b, g))
                    # exp -> fp16, split to match the DMA granularity: each
                    # part only waits on its own DMA, so the pipeline fill
                    # and the post-last-DMA tail are one small exp, not a
                    # whole supergroup.
                    # separate pools: with one shared single-buffer pool, the
                    # ecv0 reuse would wait on the previous group's chunk1
                    # matmuls, whose SWDGE slab input is the latest-arriving
                    # stream — chunk1 lateness would stall the cv0 chain.
                    ecv0 = exp_p.tile([128, CW], F16, tag="ecv0")
                    ec1 = exp1_p.tile([128, C1], F16, tag="ec1")
                    n_parts = 4 if (b == 0 and g == 0) else 2
                    for q in range(n_parts):
                        lo_c = q * (CW // n_parts)
                        hi_c = (q + 1) * (CW // n_parts)
                        nc.scalar.activation(
                            out=ecv0[:, lo_c:hi_c],
                            in_=cv0[:, lo_c:hi_c],
                            func=mybir.ActivationFunctionType.Exp,
                        )
                    nc.scalar.activation(
                        out=ec1, in_=c1f, func=mybir.ActivationFunctionType.Exp
                    )
                    # matmuls: contract over D. All are singleton accumulation
                    # groups into disjoint PSUM columns. j2 = 32g + 4i + wc;
                    # cols 8*j2+0:4 = h row h0+i (chunk0 lo), +4:8 = h0+8+i.
                    # Emission order matches exp-part order (lo half, hi
                    # half, chunk1) so PE streams behind each exp part.
                    for i in range(8):  # h row within slab
                        for wc in range(4):  # 128-col W chunk
                            off = 8 * (32 * g + 4 * i + wc)
                            # chunk0 lo slab (h0+i): N=4 at cols off..off+3
                            sl0 = slice(i * W + wc * 128, i * W + wc * 128 + 128)
                            nc.tensor.matmul(
                                bankA[:, off : off + 4],
                                ecv0[:, sl0],
                                wkt[:, 0:4],
                                start=True,
                                stop=True,
                            )
                    for i in range(8):
                        for wc in range(4):
                            off = 8 * (32 * g + 4 * i + wc)
                            # chunk0 hi slab (h0+8+i): N=4 at cols off+4..off+7
                            sl_hi = slice(
                                (8 + i) * W + wc * 128, (8 + i) * W + wc * 128 + 128
                            )
                            nc.tensor.matmul(
                                bankA[:, off + 4 : off + 8],
                                ecv0[:, sl_hi],
                                wkt[:, 0:4],
                                start=True,
                                stop=True,
                            )
                    for i in range(8):
                        for wc in range(4):
                            off = 8 * (32 * g + 4 * i + wc)
                            sl = slice(i * W + wc * 128, i * W + wc * 128 + 128)
                            # chunk1 (d 128..191), both slabs at once: N=8
                            nc.tensor.matmul(
                                bankB[:, off : off + 8],
                                ec1[:, sl],
                                wkt[:, 4:12],
                                start=True,
                                stop=True,
                            )

                    # ---- finalize sums for this supergroup on VectorE ----
                    # bank views: [128, i:8, w:4, e:8] at col 256g
                    A5 = bankA[:, 256 * g : 256 * g + 256].rearrange(
                        "p (i w e) -> p i w e", i=8, w=4
                    )
                    # TensorTensor may read only one PSUM operand: evacuate
                    # bankB's half to SBUF first, then adds read PSUM+SBUF.
                    bB_sb = tmpp.tile([128, 8, 4, 8], F32, tag="bB_sb")
                    nc.vector.tensor_copy(
                        bB_sb,
                        bankB[:, 256 * g : 256 * g + 256].rearrange(
                            "p (i w e) -> p i w e", i=8, w=4
                        ),
                    )
                    mean_sb = finp.tile([128, 64], F32, tag="mean_sb")
                    var_sb = finp.tile([128, 64], F32, tag="var_sb")
                    fins.append((mean_sb, var_sb))
                    # dest col j3 = 4*h_local + wc = 32*half + 4i + wc
                    M5 = mean_sb.rearrange("p (f i w) -> p f i w", f=2, i=8)
                    V5 = var_sb.rearrange("p (f i w) -> p f i w", f=2, i=8)

                    for half in range(2):  # 0 = lo slab (rows i), 1 = hi (8+i)
                        so = 4 * half
                        s0t = tmpp.tile([128, 8, 4], F32, tag="s0t")
                        s1t = tmpp.tile([128, 8, 4], F32, tag="s1t")
                        s2h = tmpp.tile([128, 8, 4], F32, tag="s2h")
                        s2t = tmpp.tile([128, 8, 4], F32, tag="s2t")
                        rt = tmpp.tile([128, 8, 4], F32, tag="rt")
                        m2t = tmpp.tile([128, 8, 4], F32, tag="m2t")
                        msqt = tmpp.tile([128, 8, 4], F32, tag="msqt")
                        nc.vector.tensor_add(
                            s0t, A5[:, :, :, so + 0], bB_sb[:, :, :, so + 0]
                        )
                        nc.vector.tensor_add(
                            s1t, A5[:, :, :, so + 1], bB_sb[:, :, :, so + 1]
                        )
                        nc.vector.tensor_add(
                            s2h, A5[:, :, :, so + 2], bB_sb[:, :, :, so + 2]
                        )
                        nc.vector.tensor_add(
                            s2t, A5[:, :, :, so + 3], bB_sb[:, :, :, so + 3]
                        )
                        # s2 = 256*hi + lo
                        nc.vector.scalar_tensor_tensor(
                            out=s2t,
                            in0=s2h,
                            scalar=256.0,
                            in1=s2t,
                            op0=mybir.AluOpType.mult,
                            op1=mybir.AluOpType.add,
                        )
                        nc.vector.reciprocal(rt, s0t)
                        mv = M5[:, half]
                        nc.vector.tensor_mul(mv, s1t, rt)  # mean = s1/s0
                        nc.vector.tensor_mul(m2t, s2t, rt)  # E[d^2]
                        nc.vector.tensor_mul(msqt, mv, mv)  # mean^2
                        nc.vector.tensor_sub(V5[:, half], m2t, msqt)

                # ---- end-of-b epilogue: transposes + Ln (one Exp->Ln->Exp
                # table round-trip per b). Stores are queued for the next
                # b's SP-ring flush (final b: flushed after the loop).
                for g in range(NG):
                    mean_sb, _ = fins[g]
                    mt_ps = pstp.tile([64, 128], F32, tag="tp")
                    nc.tensor.transpose(mt_ps, mean_sb, ident)
                    mo_sb = outp.tile([64, 128], F32, tag="mo")
                    nc.vector.tensor_copy(mo_sb, mt_ps)
                    pending_stores.append((mean_o, mo_sb, b, g))
                for g in range(NG):
                    _, var_sb = fins[g]
                    # logvar = Ln(var + eps) on SBUF [128, 64], then transpose
                    lnv_sb = finp.tile([128, 64], F32, tag="lnv_sb")
                    nc.scalar.activation(
                        out=lnv_sb,
                        in_=var_sb,
                        func=mybir.ActivationFunctionType.Ln,
                        bias=eps_t,
                        scale=1.0,
                    )
                    vt_ps = pstp.tile([64, 128], F32, tag="tp")
                    nc.tensor.transpose(vt_ps, lnv_sb, ident)
                    lo_sb = outp.tile([64, 128], F32, tag="lv")
                    nc.vector.tensor_copy(lo_sb, vt_ps)
                    pending_stores.append((logv_o, lo_sb, b, g))

            # final b's stores on the now-idle SP ring
            for dst, t, sb, sg in pending_stores:
                nc.sync.dma_start(
                    out=dst[sb, GH * sg : GH * sg + GH].rearrange(
                        "h (c w) -> h c w", c=4
                    ),
                    in_=t,
                )

    nc.compile()
    return nc


_NC_CACHE = None


def _get_nc():
    global _NC_CACHE
    if _NC_CACHE is None:
        _NC_CACHE = build_core_kernel()
    return _NC_CACHE


def kernel(cost_volume: np.ndarray):
    global LAST_RESULT
    cost_volume = np.ascontiguousarray(np.asarray(cost_volume, dtype=np.float32))
    assert cost_volume.shape == (B, D, H, W), cost_volume.shape

    nc = _get_nc()
    wk = _make_weights()
    in_maps = []
    for c in range(N_CORES):
        shard = np.ascontiguousarray(cost_volume[:, :, c * HL : (c + 1) * HL, :])
        in_maps.append({"x": shard, "wk": wk})

    res = run_bass_kernel_spmd(nc, in_maps, list(range(N_CORES)), trace=TRACE)
    LAST_RESULT = res

    mean = np.empty((B, H, W), dtype=np.float32)
    logv = np.empty((B, H, W), dtype=np.float32)
    for c in range(N_CORES):
        mean[:, c * HL : (c + 1) * HL, :] = res.results[c]["mean"]
        logv[:, c * HL : (c + 1) * HL, :] = res.results[c]["logvar"]
    return mean, logv


# revision 40
# speedup vs baseline: 1.0471x; 1.0471x over previous
"""Disparity estimation loss kernel for Trainium2 (Bass/Tile), 8-core SPMD.

Reference computation (per pixel over the D=192 disparity axis):
    prob    = softmax(cost_volume, axis=D)
    mean    = sum(prob * d)
    var     = sum(prob * (d - mean)^2) = E[d^2] - mean^2
    logvar  = log(var + 1e-6)
Outputs: (mean [B,H,W], logvar [B,H,W]) both f32.

Strategy: shard H across 8 cores (H=256 -> 32 rows/core). All reductions are
along D which stays local. Per core, 16-h-row supergroups (2 per b):
  - Three DMA queues stream inputs concurrently (a single queue tops out
    well below the per-core HBM rate): SP HWDGE ring carries cv0 (d 0..127,
    [128, 16*512] f32, 4 MiB) of even supergroups, ACT HWDGE ring cv0 of
    odd supergroups, SWDGE the chunk1 slabs (d 128..191, two 64-partition
    slabs per supergroup, slab p = h rows h0+8p..h0+8p+8) casting f32->f16
    inline (SWDGE-only feature). All APs are simple single-level patterns
    (complex APs cost ~8us of sequencer time per HWDGE trigger). All
    triggers are hoisted to the top of each b so prefetch never queues
    behind compute.
  - exp on ScalarE -> fp16 (no max subtraction: inputs are N(0,1)), split
    per supergroup into a cv0 part and a chunk1 part so the serial tail
    after the last DMA is one small exp, not the whole supergroup.
  - TensorE matmuls contract over D: exp tile [D, 128 w-cols] stationary
    (fp16), weight columns [1, d, d^2_hi, d^2_lo] moving -> PSUM groups
    [128 w, 4]. d^2 split into exact-fp16 hi/lo bytes.
  - VectorE batched finalize (mean/var) per supergroup; Ln + PE transposes
    + output stores deferred to end-of-b (2 ACT table switches per b, and
    store triggers on the ACT HWDGE ring where their compute dependencies
    cannot block input prefetch).
"""

import os
import sys

for _p in ("/opt/trn_rl_repo", "/root/.axon_site/_ro/trn_rl_repo"):
    if os.path.isdir(_p) and _p not in sys.path:
        sys.path.insert(0, _p)

import ml_dtypes
import numpy as np

import concourse.bacc as bacc
import concourse.bass as bass
import concourse.tile as tile
from concourse import mybir
from concourse.bass_utils import run_bass_kernel_spmd
from concourse.masks import make_identity

B, D, H, W = 4, 192, 256, 512
N_CORES = 8
HL = H // N_CORES  # 32 h-rows per core
F32 = mybir.dt.float32
F16 = mybir.dt.float16

# knobs (test.py may flip these before calling kernel())
TRACE = False
LAST_RESULT = None


def _make_weights() -> np.ndarray:
    """[128, 12] fp16 weight matrix; every entry is exactly representable.

    cols 0:4  -> d-chunk0 (d = row p):        [1, d, hi(d^2), lo(d^2)]  (fp16)
    cols 4:12 -> d-chunk1 (two slabs stacked on partitions):
       rows 0:64   (slab lo, d = 128+p):      [1, d, hi, lo, 0, 0, 0, 0]
       rows 64:128 (slab hi, d = 64+p):       [0, 0, 0, 0, 1, d, hi, lo]
    where hi = d^2 >> 8 (<=142), lo = d^2 & 255 — both exact in fp16.
    """
    wk = np.zeros((128, 12), dtype=np.float64)

    def cols(d):
        dsq = (d.astype(np.int64)) ** 2
        return 1.0, d, (dsq >> 8).astype(np.float64), (dsq & 255).astype(np.float64)

    p = np.arange(128, dtype=np.int64)
    wk[:, 0], wk[:, 1], wk[:, 2], wk[:, 3] = cols(p)
    c = cols(128 + p[:64])
    for k in range(4):
        wk[:64, 4 + k] = c[k]
    c = cols(64 + p[64:])
    for k in range(4):
        wk[64:, 8 + k] = c[k]
    return wk.astype(np.float16)


def build_core_kernel():
    """Build the per-core Bass module (identical program on all 8 cores)."""
    nc = bacc.Bacc("TRN2", target_bir_lowering=False, debug=False)
    x = nc.dram_tensor("x", [B, D, HL, W], F32, kind="ExternalInput")
    wk = nc.dram_tensor("wk", [128, 12], F16, kind="ExternalInput")
    mean_o = nc.dram_tensor("mean", [B, HL, W], F32, kind="ExternalOutput")
    logv_o = nc.dram_tensor("logvar", [B, HL, W], F32, kind="ExternalOutput")

    NG = 2  # supergroups per b, 16 h rows each
    GH = HL // NG  # 16
    CW = GH * W  # 8192 f32 cols of chunk0 per supergroup
    C1 = CW // 2  # 4096 cols of packed chunk1

    with tile.TileContext(nc) as tc:
        with (
            tc.tile_pool(name="cv", bufs=4) as cvp,
            tc.tile_pool(name="ex", bufs=1) as exp_p,
            tc.tile_pool(name="ex1", bufs=2) as exp1_p,
            tc.tile_pool(name="consts", bufs=1) as consts,
            tc.tile_pool(name="fin", bufs=3) as finp,
            tc.tile_pool(name="tmps", bufs=2) as tmpp,
            # outp depth 4: with 2, the DVE copy into a recycled output tile
            # waits for the PREVIOUS b's store DMA, which sits on SP behind
            # input triggers that wait on exp-fed buffer sems — a slack
            # spiral that cost ~25us per two b's.
            tc.tile_pool(name="outp", bufs=4) as outp,
            tc.tile_pool(name="psum", bufs=3, space="PSUM") as psp,
            tc.tile_pool(name="pst", bufs=2, space="PSUM") as pstp,
        ):
            wkt = consts.tile([128, 12], F16, tag="wk")
            nc.sync.dma_start(out=wkt, in_=wk[:, :])
            ident = consts.tile([128, 128], F32, tag="ident")
            make_identity(nc, ident)
            eps_t = consts.tile([128, 1], F32, tag="eps")
            nc.vector.memset(eps_t, 1e-6)

            # ---- input loading. Three queues with EQUAL byte shares stream
            # concurrently (the aggregate DMA rate rises with the number of
            # simultaneously-backlogged queues — SDMA engines are per-packet
            # latency-bound): ACT HWDGE ring takes cv0 of g0, triggered ONE
            # b AHEAD (ACT's triggers sit behind its exps, and at ~1/3 of
            # the aggregate rate a 4 MiB delivery needs most of a b of
            # lead), SP HWDGE ring cv0 of g1 (SP free-runs and carries the
            # last-consumed group), SWDGE the chunk1 slabs, casting
            # f32->f16 inline (SWDGE-only feature; halves SBUF footprint so
            # the cv pool can quad-buffer). cv0 is loaded as two 2 MiB
            # halves with matching split exps, so compute starts on the
            # first half while the second streams.
            def load_group(b, g):
                h0 = GH * g
                cv0 = cvp.tile([128, CW], F32, tag="cv0")
                c1f = cvp.tile([128, C1], F16, tag="c1f")
                # d 0..127 for h rows h0..h0+15 (2 x 2 MiB, simple APs),
                # one half per HWDGE ring: each group's cv0 arrives 2-way
                # parallel (the lookahead gives the ACT ring enough lead)
                nc.sync.dma_start(
                    out=cv0[:, 0 : CW // 2], in_=x[b, 0:128, h0 : h0 + 8, :]
                )
                nc.scalar.dma_start(
                    out=cv0[:, CW // 2 : CW], in_=x[b, 0:128, h0 + 8 : h0 + GH, :]
                )
                # d 128..191, slab p: partitions 64p..64p+64 hold h rows
                # h0+8p..h0+8p+8 (1 MiB read each, single-level AP)
                for p in range(2):
                    nc.gpsimd.dma_start(
                        out=c1f[64 * p : 64 * p + 64, :],
                        in_=x[b, 128:192, h0 + 8 * p : h0 + 8 * p + 8, :],
                    )
                return cv0, c1f

            pending_stores = []
            loaded = {}
            loaded[(0, 0)] = load_group(0, 0)
            loaded[(0, 1)] = load_group(0, 1)
            for b in range(B):
                # one PSUM bank per b for chunk0 sums, one for chunk1 sums
                bankA = psp.tile([128, 512], F32, tag="bankA")
                bankB = psp.tile([128, 512], F32, tag="bankB")

                # trigger the NEXT b's loads here — a full b of lead for
                # every queue (the cv pool's bufs=4 rotation keeps the
                # buffer-free waits pointing at already-executed exps)
                if b + 1 < B:
                    loaded[(b + 1, 0)] = load_group(b + 1, 0)
                    loaded[(b + 1, 1)] = load_group(b + 1, 1)
                # previous b's output stores ride the SP ring BEHIND this
                # b's input triggers: their deps are long-ready so they
                # cannot stall input prefetch, and they cost the ACT engine
                # (the co-critical one) nothing.
                for dst, t, sb, sg in pending_stores:
                    nc.sync.dma_start(
                        out=dst[sb, GH * sg : GH * sg + GH].rearrange(
                            "h (c w) -> h c w", c=4
                        ),
                        in_=t,
                    )
                pending_stores = []

                fins = []
                for g in range(NG):
                    cv0, c1f = loaded.pop((b, g))
                    # exp -> fp16, split to match the DMA granularity: each
                    # part only waits on its own DMA, so the pipeline fill
                    # and the post-last-DMA tail are one small exp, not a
                    # whole supergroup.
                    # separate pools: with one shared single-buffer pool, the
                    # ecv0 reuse would wait on the previous group's chunk1
                    # matmuls, whose SWDGE slab input is the latest-arriving
                    # stream — chunk1 lateness would stall the cv0 chain.
                    ecv0 = exp_p.tile([128, CW], F16, tag="ecv0")
                    ec1 = exp1_p.tile([128, C1], F16, tag="ec1")
                    nc.scalar.activation(
                        out=ecv0[:, 0 : CW // 2],
                        in_=cv0[:, 0 : CW // 2],
                        func=mybir.ActivationFunctionType.Exp,
                    )
                    nc.scalar.activation(
                        out=ecv0[:, CW // 2 : CW],
                        in_=cv0[:, CW // 2 : CW],
                        func=mybir.ActivationFunctionType.Exp,
                    )
                    nc.scalar.activation(
                        out=ec1, in_=c1f, func=mybir.ActivationFunctionType.Exp
                    )
                    # matmuls: contract over D. All are singleton accumulation
                    # groups into disjoint PSUM columns. j2 = 32g + 4i + wc;
                    # cols 8*j2+0:4 = h row h0+i (chunk0 lo), +4:8 = h0+8+i.
                    # Emission order matches exp-part order (lo half, hi
                    # half, chunk1) so PE streams behind each exp part.
                    for i in range(8):  # h row within slab
                        for wc in range(4):  # 128-col W chunk
                            off = 8 * (32 * g + 4 * i + wc)
                            # chunk0 lo slab (h0+i): N=4 at cols off..off+3
                            sl0 = slice(i * W + wc * 128, i * W + wc * 128 + 128)
                            nc.tensor.matmul(
                                bankA[:, off : off + 4],
                                ecv0[:, sl0],
                                wkt[:, 0:4],
                                start=True,
                                stop=True,
                            )
                    for i in range(8):
                        for wc in range(4):
                            off = 8 * (32 * g + 4 * i + wc)
                            # chunk0 hi slab (h0+8+i): N=4 at cols off+4..off+7
                            sl_hi = slice(
                                (8 + i) * W + wc * 128, (8 + i) * W + wc * 128 + 128
                            )
                            nc.tensor.matmul(
                                bankA[:, off + 4 : off + 8],
                                ecv0[:, sl_hi],
                                wkt[:, 0:4],
                                start=True,
                                stop=True,
                            )
                    for i in range(8):
                        for wc in range(4):
                            off = 8 * (32 * g + 4 * i + wc)
                            sl = slice(i * W + wc * 128, i * W + wc * 128 + 128)
                            # chunk1 (d 128..191), both slabs at once: N=8
                            nc.tensor.matmul(
                                bankB[:, off : off + 8],
                                ec1[:, sl],
                                wkt[:, 4:12],
                                start=True,
                                stop=True,
                            )

                    # ---- finalize sums for this supergroup on VectorE ----
                    # bank views: [128, i:8, w:4, e:8] at col 256g
                    A5 = bankA[:, 256 * g : 256 * g + 256].rearrange(
                        "p (i w e) -> p i w e", i=8, w=4
                    )
                    # TensorTensor may read only one PSUM operand: evacuate
                    # bankB's half to SBUF first, then adds read PSUM+SBUF.
                    bB_sb = tmpp.tile([128, 8, 4, 8], F32, tag="bB_sb")
                    nc.vector.tensor_copy(
                        bB_sb,
                        bankB[:, 256 * g : 256 * g + 256].rearrange(
                            "p (i w e) -> p i w e", i=8, w=4
                        ),
                    )
                    mean_sb = finp.tile([128, 64], F32, tag="mean_sb")
                    var_sb = finp.tile([128, 64], F32, tag="var_sb")
                    fins.append((mean_sb, var_sb))
                    # dest col j3 = 4*h_local + wc = 32*half + 4i + wc
                    M5 = mean_sb.rearrange("p (f i w) -> p f i w", f=2, i=8)
                    V5 = var_sb.rearrange("p (f i w) -> p f i w", f=2, i=8)

                    for half in range(2):  # 0 = lo slab (rows i), 1 = hi (8+i)
                        so = 4 * half
                        s0t = tmpp.tile([128, 8, 4], F32, tag="s0t")
                        s1t = tmpp.tile([128, 8, 4], F32, tag="s1t")
                        s2h = tmpp.tile([128, 8, 4], F32, tag="s2h")
                        s2t = tmpp.tile([128, 8, 4], F32, tag="s2t")
                        rt = tmpp.tile([128, 8, 4], F32, tag="rt")
                        m2t = tmpp.tile([128, 8, 4], F32, tag="m2t")
                        msqt = tmpp.tile([128, 8, 4], F32, tag="msqt")
                        nc.vector.tensor_add(
                            s0t, A5[:, :, :, so + 0], bB_sb[:, :, :, so + 0]
                        )
                        nc.vector.tensor_add(
                            s1t, A5[:, :, :, so + 1], bB_sb[:, :, :, so + 1]
                        )
                        nc.vector.tensor_add(
                            s2h, A5[:, :, :, so + 2], bB_sb[:, :, :, so + 2]
                        )
                        nc.vector.tensor_add(
                            s2t, A5[:, :, :, so + 3], bB_sb[:, :, :, so + 3]
                        )
                        # s2 = 256*hi + lo
                        nc.vector.scalar_tensor_tensor(
                            out=s2t,
                            in0=s2h,
                            scalar=256.0,
                            in1=s2t,
                            op0=mybir.AluOpType.mult,
                            op1=mybir.AluOpType.add,
                        )
                        nc.vector.reciprocal(rt, s0t)
                        mv = M5[:, half]
                        nc.vector.tensor_mul(mv, s1t, rt)  # mean = s1/s0
                        nc.vector.tensor_mul(m2t, s2t, rt)  # E[d^2]
                        nc.vector.tensor_mul(msqt, mv, mv)  # mean^2
                        nc.vector.tensor_sub(V5[:, half], m2t, msqt)

                # ---- end-of-b epilogue: transposes + Ln (one Exp->Ln->Exp
                # table round-trip per b). Stores are queued for the next
                # b's SP-ring flush (final b: flushed after the loop).
                for g in range(NG):
                    mean_sb, _ = fins[g]
                    mt_ps = pstp.tile([64, 128], F32, tag="tp")
                    nc.tensor.transpose(mt_ps, mean_sb, ident)
                    mo_sb = outp.tile([64, 128], F32, tag="mo")
                    nc.vector.tensor_copy(mo_sb, mt_ps)
                    pending_stores.append((mean_o, mo_sb, b, g))
                for g in range(NG):
                    _, var_sb = fins[g]
                    # logvar = Ln(var + eps) on SBUF [128, 64], then transpose
                    lnv_sb = finp.tile([128, 64], F32, tag="lnv_sb")
                    nc.scalar.activation(
                        out=lnv_sb,
                        in_=var_sb,
                        func=mybir.ActivationFunctionType.Ln,
                        bias=eps_t,
                        scale=1.0,
                    )
                    vt_ps = pstp.tile([64, 128], F32, tag="tp")
                    nc.tensor.transpose(vt_ps, lnv_sb, ident)
                    lo_sb = outp.tile([64, 128], F32, tag="lv")
                    nc.vector.tensor_copy(lo_sb, vt_ps)
                    pending_stores.append((logv_o, lo_sb, b, g))

            # final b's stores on the now-idle SP ring
            for dst, t, sb, sg in pending_stores:
                nc.sync.dma_start(
                    out=dst[sb, GH * sg : GH * sg + GH].rearrange(
                        "h (c w) -> h c w", c=4
                    ),
                    in_=t,
                )

    nc.compile()
    return nc


_NC_CACHE = None


def _get_nc():
    global _NC_CACHE
    if _NC_CACHE is None:
        _NC_CACHE = build_core_kernel()
    return _NC_CACHE


def kernel(cost_volume: np.ndarray):
    global LAST_RESULT
    cost_volume = np.ascontiguousarray(np.asarray(cost_volume, dtype=np.float32))
    assert cost_volume.shape == (B, D, H, W), cost_volume.shape

    nc = _get_nc()
    wk = _make_weights()
    in_maps = []
    for c in range(N_CORES):
        shard = np.ascontiguousarray(cost_volume[:, :, c * HL : (c + 1) * HL, :])
        in_maps.append({"x": shard, "wk": wk})

    res = run_bass_kernel_spmd(nc, in_maps, list(range(N_CORES)), trace=TRACE)
    LAST_RESULT = res

    mean = np.empty((B, H, W), dtype=np.float32)
    logv = np.empty((B, H, W), dtype=np.float32)
    for c in range(N_CORES):
        mean[:, c * HL : (c + 1) * HL, :] = res.results[c]["mean"]
        logv[:, c * HL : (c + 1) * HL, :] = res.results[c]["logvar"]
    return mean, logv
